# revision 67
# baseline (speedup 1.0000x reference)
"""Causal multi-head attention (B=4, S=2048, D=1024, H=16) on 8 TRN2 cores.

Sharding (DP on batch x TP on heads): core 2b+g handles batch b and heads
8g..8g+8.  Each core computes the qkv projection for its head group, causal
attention, and a partial output projection; the host sums the two partials
per batch and undoes the fixed power-of-2 scaling -- no device collectives.

v2: fp8 DoubleRow matmuls for the qkv projections.  The qkv weights are
pre-scaled by 256 and shipped as fp8e4 (e4m3) hi/lo pairs; x ships as an
fp8e4 hi/lo pair (x8 + rx8 residual).  q/k/v are computed with a 3-pass
compensated fp8 product (x8*w8 + rx8*w8 + x8*rw8, all DoubleRow with two
128-deep contraction planes per instruction) which matches fp16 precision
at 3/4 (q/k) and 3/4 (v) of the fp16 PE cost.  Scores and pv stay
fp16/bf16 (fp8 probs would need flash-style per-query max subtraction:
unnormalized exp(s) reaches e^18, far past e4m3's 240 max), and the
out-projection stays fp16 (the fp8 quantization of on-device a cannot be
compensated without extra DVE passes and measures 4e-2 max-rel-err).

Scale bookkeeping costs nothing: q/k/v evictions fold 1/256 into the DVE
psum->sbuf copy that already existed (biases are zero, so the bias-add
becomes a scalar-mul of the same cost).  The out-projection is evicted
as bf16 (halves the output DMA bytes) and the host sums/upcasts the two
per-batch partials.

v3 scheduling (cost-model driven, PE busy ~210us of 239us total): the PE
executes its in-order stream, so emission order is the schedule.  The hp0
qkv runs as a 6-group split-pass ramp (pass/j-major, matched to the
serial-DMA arrival order of the fp8 inputs); each later head pair's qkv
groups are emitted into the previous head pair's attention chunk windows;
and each chunk's DIAGONAL score group + exp is emitted inside the
previous chunk (just before its pv) so the pv never waits on ACT and the
exp pipeline always runs one chunk ahead.  The causal mask constant is
DMA'd from the host (gpsimd affine_select would cost ~23us of Pool
time), stage3 uses ps1 during attention and fans out over all psum pools
for the final chunk, whose evictions go to the by-then-idle ACT engine.

Everything else follows the v1 design: layouts avoid all on-device
transposes, block-causal skipping at 128 granularity with column-sliced
diagonal tiles, exp without max-subtraction, denominator via the pv ones
column, reciprocal+broadcast-DMA normalize chain off the PE critical path,
qkv/attention interleaving per head pair, and PSUM-bank phase borrowing
during the startup DMA ramp.
"""

import numpy as np

import concourse.bass as bass
import concourse.mybir as mybir
from concourse import bacc
from concourse.bass import ds
from concourse.tile import TileContext

F16 = mybir.dt.float16
F32 = mybir.dt.float32
BF16 = mybir.dt.bfloat16
F8 = mybir.dt.float8e4

S = 2048  # sequence length
D = 1024  # model dim
HD = 64  # head dim
HPC = 8  # heads per core
GD = HPC * HD  # 512, per-core qkv width
N_CORES = 8

WS = 256.0  # weight prescale (fp8 dynamic range)

AF = mybir.ActivationFunctionType
ALU = mybir.AluOpType
DR = mybir.MatmulPerfMode.DoubleRow


def build_bass(nloop=1):
    nc = bacc.Bacc(None, target_bir_lowering=False)

    x8_d = nc.dram_tensor("x8", [D, S], F8, kind="ExternalInput")
    rx8_d = nc.dram_tensor("rx8", [D, S], F8, kind="ExternalInput")
    wq8_d = nc.dram_tensor("wq8", [D, GD], F8, kind="ExternalInput")
    rwq8_d = nc.dram_tensor("rwq8", [D, GD], F8, kind="ExternalInput")
    wk8_d = nc.dram_tensor("wk8", [D, GD], F8, kind="ExternalInput")
    rwk8_d = nc.dram_tensor("rwk8", [D, GD], F8, kind="ExternalInput")
    wv8_d = nc.dram_tensor("wv8", [D, GD], F8, kind="ExternalInput")
    rwv8_d = nc.dram_tensor("rwv8", [D, GD], F8, kind="ExternalInput")
    wp_d = nc.dram_tensor("wp", [GD, D], F16, kind="ExternalInput")
    masks_d = nc.dram_tensor("masks", [128, 1280], BF16, kind="ExternalInput")
    out_d = nc.dram_tensor("out", [S, D], BF16, kind="ExternalOutput")

    with TileContext(nc) as tc:
     for _loop in range(nloop):
      with tc.tile_pool(name="persist", bufs=1) as persist:
        # Per-head-pair q/k (transposed [douts, rows]; partitions 0:64 =
        # even head dims, 64:128 = odd head dims) and v (natural [keys,
        # per-pair 2*65] with a ones column per head at local col 64 so the
        # pv matmul emits the softmax denominator as row 64).
        qTs, kTs, vs = [], [], []
        for hp in range(4):
            qrow, krow = [], []
            for n in range(4):
                t_q = persist.tile([128, 512], F16, tag=f"qT{hp}_{n}")
                t_k = persist.tile([128, 512], F16, tag=f"kT{hp}_{n}")
                qrow.append(t_q)
                krow.append(t_k)
            vrow = []
            for g in range(4):
                t_v = persist.tile([128, 4 * 130], BF16, tag=f"v{hp}_{g}")
                vrow.append(t_v)
            qTs.append(qrow)
            kTs.append(krow)
            vs.append(vrow)
        wp_sb = persist.tile([128, 4 * D], F16)

        # aT per-chunk tiles: aTc[c] = [128, 4*512], columns hp-major
        # (hp*512 + q-within-chunk); partitions = head-pair dm packing
        aTc = []
        for c in range(4):
            aTc_t = persist.tile([128, 4 * 512], F16, tag=f"aTc{c}")
            aTc.append(aTc_t)

        # Precomputed causal mask tiles (host-built constant: gpsimd
        # affine_select costs ~23us of in-order Pool time and delays every
        # gpsimd-issued DMA behind it): for diagonal offset d = j*128 only
        # columns [d:512) are ever used, and in that sliced frame the
        # triangle is mask[i, qq] = 1 if qq >= i else 0.
        MOFF = [0, 512, 896, 1152]  # packed offsets, widths 512-128j
        masks = persist.tile([128, 1280], BF16)

        with (
            tc.tile_pool(name="stage1", bufs=1) as s1,
            tc.tile_pool(name="probs", bufs=3) as probp,
            tc.tile_pool(name="small", bufs=2) as smallp,
            tc.tile_pool(name="outp", bufs=8) as outp,
            tc.tile_pool(name="ps1", bufs=2, space="PSUM") as ps1,
            tc.tile_pool(name="ps_sc", bufs=2, space="PSUM") as ps_sc,
            tc.tile_pool(name="ps_pv", bufs=2, space="PSUM") as ps_pv,
        ):
            # Input tiles hold DoubleRow kt-plane pairs: tile j's columns
            # [0:W] are contraction rows 2j*128..2j*128+127, [W:2W] are rows
            # (2j+1)*128...  DMAs split across the sync/scalar/gpsimd issue
            # queues, ordered by first use: wq8 + x8 first (pass A of the
            # first q groups), then the residuals, then wk8/wv8/wp8.
            def pair_load(dram, j, width, tile, eng):
                eng.dma_start(
                    out=tile[:, :].rearrange("p (two s) -> p two s", two=2),
                    in_=dram[2 * j * 128 : (2 * j + 2) * 128, :].rearrange(
                        "(two p) s -> p two s", two=2
                    ),
                )

            # arrival order tuned to the hp0 split-pass ramp: w tensors +
            # x8 first (A/C passes), rx8 (B) after, then the v/proj
            # weights.  Each weight tensor is a single DMA (one
            # descriptor-generation stage instead of four -- the dge
            # stages serialize and dominate the small w transfers); x8
            # rides the SWDGE queue so its dge doesn't queue behind the
            # weights' HWDGE stages.
            def w_load(dram, name, eng):
                W = dram.shape[1]
                t = s1.tile([128, 8 * W], F8, tag=name, name=name + "t")
                eng.dma_start(
                    out=t[:, :].rearrange(
                        "p (four two c) -> p four two c", four=4, two=2
                    ),
                    in_=dram[:, :].rearrange(
                        "(four two p) c -> p four two c", four=4, two=2
                    ),
                )
                return t

            sy, sc_, gp = nc.sync, nc.scalar, nc.gpsimd
            # issue interleaved x8[j]/wq8[j] pairs so the serial DMA device
            # delivers them in the exact order the hp0 A-pass consumes them
            wq8t = w_load(wq8_d, "wq8", sy)
            rwq8t = w_load(rwq8_d, "rwq8", sy)
            wk8t = w_load(wk8_d, "wk8", sy)
            rwk8t = w_load(rwk8_d, "rwk8", sy)
            x8t, rx8t = [None] * 4, [None] * 4
            for j in range(4):
                t = s1.tile([128, 2 * S], F8, tag=f"x8{j}", name=f"x8s{j}")
                pair_load(x8_d, j, S, t, gp)
                x8t[j] = t
            for j in range(4):
                t = s1.tile([128, 2 * S], F8, tag=f"rx8{j}", name=f"rx8s{j}")
                pair_load(rx8_d, j, S, t, sc_)
                rx8t[j] = t
            nc.gpsimd.dma_start(out=masks[:, :], in_=masks_d[:, :])
            wv8t = w_load(wv8_d, "wv8", gp)
            rwv8t = w_load(rwv8_d, "rwv8", gp)
            for k in range(4):
                nc.sync.dma_start(
                    out=wp_sb[:, ds(k * D, D)],
                    in_=wp_d[k * 128 : (k + 1) * 128, :],
                )

            def w_planes(t, j, hp):
                return t[:, :].rearrange(
                    "p (four two c) -> p four two c", four=4, two=2
                )[:, j, :, ds(hp * 128, 128)]

            def x_planes(tiles, j, c0, w):
                return tiles[j][:, :].rearrange(
                    "p (two s) -> p two s", two=2
                )[:, :, ds(c0, w)]

            # one qkv projection group: 3-pass compensated fp8 DoubleRow
            # (A: x8*w8, C: x8*rw8, B: rx8*w8 -- B last so the rx8 DMAs
            # are off the startup critical path).
            def emit_qk_group(hp, wt, rwt, dst, n, ps):
                passes = ((wt, x8t), (rwt, x8t), (wt, rx8t))
                for pi, (lw, lx) in enumerate(passes):
                    for j in range(4):
                        nc.tensor.matmul(
                            ps[:, :],
                            w_planes(lw, j, hp),
                            x_planes(lx, j, n * 512, 512),
                            start=(pi == 0 and j == 0),
                            stop=(pi == 2 and j == 3),
                            perf_mode=DR,
                        )
                nc.vector.tensor_scalar_mul(
                    out=dst[n][:, :], in0=ps[:, :], scalar1=1.0 / WS
                )

            # hp0 startup: 6 psum groups held open (q n0..3 + k n0,n1,
            # borrowing the idle scores/pv banks), emitted pass-major and
            # j-major so each arriving DMA tile unlocks a batch of matmuls
            # and PE ramps as the inputs trickle in.
            def emit_qk0():
                qps = [
                    ps1.tile([128, 512], F32, tag="ps", name="qps0"),
                    ps1.tile([128, 512], F32, tag="ps", name="qps1"),
                    ps_sc.tile([128, 512], F32, tag="sc", name="qps2"),
                    ps_sc.tile([128, 512], F32, tag="sc", name="qps3"),
                ]
                # p-state warmers: zero-data K=1 matmuls through the pv
                # psum slots keep the PE ramp alive through the startup
                # DMA wait, so the real split-pass matmuls run at the full
                # 2.4GHz cycle instead of the mid p-state.
                zf = smallp.tile([1, 512], F16, name="zf")
                nc.vector.memset(zf[:, :], 0.0)
                fill_tiles = [
                    ps_pv.tile([128, 512], F32, tag="pvpo", name=f"fill{i}")
                    for i in range(26)
                ]
                fill_n = [0]

                def emit_fill(count):
                    for _ in range(count):
                        if fill_n[0] >= len(fill_tiles):
                            return
                        nc.tensor.matmul(
                            fill_tiles[fill_n[0]][:, :],
                            zf[0:1, 0:128],
                            zf[0:1, 0:512],
                            start=True, stop=True,
                        )
                        fill_n[0] += 1

                emit_fill(14)
                kps = [
                    ps_pv.tile([128, 512], F32, tag="pvpo", name="kps0"),
                    ps_pv.tile([128, 512], F32, tag="pvpo", name="kps1"),
                ]
                groups = [("q", n, qps[n]) for n in range(4)] + [
                    ("k", n, kps[n]) for n in range(2)
                ]

                def w_of(kind, res):
                    if kind == "q":
                        return rwq8t if res else wq8t
                    return rwk8t if res else wk8t

                qg = [g for g in groups if g[0] == "q"]
                kg = [g for g in groups if g[0] == "k"]
                sched = [s for j in range(4)
                         for s in ((0, False, x8t, j, qg),
                                   (1, True, x8t, j, qg))]
                sched += [s for j in range(4)
                          for s in ((0, False, x8t, j, kg),
                                    (1, True, x8t, j, kg))]
                sched += [(2, False, rx8t, j, qg) for j in range(4)]
                sched += [(2, False, rx8t, j, kg) for j in range(4)]
                for pi, res, lx, j, gset in sched:
                    for kind, n, ps in gset:
                        nc.tensor.matmul(
                            ps[:, :],
                            w_planes(w_of(kind, res), j, 0),
                            x_planes(lx, j, n * 512, 512),
                            start=(pi == 0 and j == 0),
                            stop=(pi == 2 and j == 3),
                            perf_mode=DR,
                        )
                    if pi != 2:
                        emit_fill(2)
                emit_fill(len(fill_tiles))
                for kind, n, ps in groups:
                    dst = qTs[0] if kind == "q" else kTs[0]
                    nc.vector.tensor_scalar_mul(
                        out=dst[n][:, :], in0=ps[:, :], scalar1=1.0 / WS
                    )
                for n in (2, 3):
                    ps = ps1.tile([128, 512], F32, tag="ps")
                    emit_qk_group(0, wk8t, rwk8t, kTs[0], n, ps)

            def emit_vgrp(hp, g):
                # v rows for key tiles 4g..4g+3 of head pair hp (3-pass
                # compensated fp8 DoubleRow); eviction folds the 1/WS
                # descale.
                for rl in range(4):
                    rt = 4 * g + rl
                    ps = ps1.tile([128, 512], F32, tag="ps")
                    passes = ((x8t, wv8t), (rx8t, wv8t), (x8t, rwv8t))
                    for pi, (lx, lw) in enumerate(passes):
                        for j in range(4):
                            nc.tensor.matmul(
                                ps[0:128, 0:128],
                                x_planes(lx, j, rt * 128, 128),
                                w_planes(lw, j, hp),
                                start=(pi == 0 and j == 0),
                                stop=(pi == 2 and j == 3),
                                perf_mode=DR,
                            )
                    # interleaved store: local head hl -> cols
                    # [hl*65, hl*65+64)
                    out_ap = vs[hp][g][:, ds(rl * 130, 130)].rearrange(
                        "p (h c) -> p h c", h=2
                    )[:, :, 0:64]
                    in_ap = ps[:, 0:128].rearrange("p (h c) -> p h c", h=2)
                    nc.vector.tensor_scalar_mul(
                        out=out_ap, in0=in_ap, scalar1=1.0 / WS
                    )
                # ones columns (softmax denominator source)
                ones_ap = vs[hp][g][:, :].rearrange("p (r c) -> p r c", c=65)[
                    :, :, 64:65
                ]
                nc.gpsimd.memset(ones_ap, 1.0)

            # out-projection for one chunk-column (all 4 aTc[c] writers
            # done); bias is zero so the eviction is a plain copy.  Both
            # psum groups come from ps1 (free during hp3's attention) so
            # stage3 never contends with the attention pv pool; the final
            # chunk's evictions go to ACT (idle once the last exps retire)
            # so the tail doesn't serialize behind DVE.
            def emit_stage3_steps(c3):
                for rt in range(4 * c3, 4 * c3 + 4):
                    for nch in range(2):
                        if c3 == 3:
                            # attention is done: every psum pool is idle
                            k3 = (2 * (rt % 4) + nch) % 3
                            if k3 == 0:
                                ps = ps1.tile([128, 512], F32, tag="ps")
                            elif k3 == 1:
                                ps = ps_pv.tile([128, 512], F32, tag="pvpo")
                            else:
                                ps = ps_sc.tile([128, 512], F32, tag="sc")
                        else:
                            ps = ps1.tile([128, 512], F32, tag="ps")
                        for kt4 in range(4):
                            nc.tensor.matmul(
                                ps[:, :],
                                aTc[c3][:, ds(kt4 * 512 + (rt % 4) * 128, 128)],
                                wp_sb[:, ds(kt4 * D + nch * 512, 512)],
                                start=(kt4 == 0),
                                stop=(kt4 == 3),
                            )
                        osb = outp.tile([128, 512], BF16, tag="osb")
                        if c3 == 3:
                            nc.scalar.copy(out=osb[:, :], in_=ps[:, :])
                        else:
                            nc.vector.tensor_copy(out=osb[:, :], in_=ps[:, :])
                        nc.sync.dma_start(
                            out=out_d[
                                rt * 128 : (rt + 1) * 128,
                                nch * 512 : (nch + 1) * 512,
                            ],
                            in_=osb[:, :],
                        )
                        yield

            def emit_stage3(c3):
                for _ in emit_stage3_steps(c3):
                    pass

            # attention for (hp, c). Even head on PE row-tile (0,0), odd head
            # on (64,0); adjacent even/odd matmuls run concurrently on the
            # two array halves.
            def emit_diag_scores_steps(hp, c, out):
                # the 2 diagonal score groups + exps of chunk (hp, c);
                # emitted from inside the PREVIOUS chunk (first group just
                # before its pv, second between the pv half-blocks so the
                # sc-pool recycle overlaps pv matmuls) -- ACT gets a head
                # start and this chunk's pv never waits on its first probs
                q0 = c * 512
                prDA = probp.tile([128, 4 * 512], BF16, tag="probsD",
                                  bufs=4)
                prDB = probp.tile([128, 4 * 512], BF16, tag="probsD",
                                  bufs=4)
                out.extend([prDA, prDB])
                for g in (4 * c, 4 * c + 2):
                    scA = ps_sc.tile([128, 1024], F32, tag="sc")
                    scB = ps_sc.tile([128, 1024], F32, tag="sc")
                    for j in (0, 1):
                        kt = g + j
                        dd = max(0, kt * 128 - q0)
                        kt_t = kTs[hp][kt // 4]
                        kcol = ds((kt % 4) * 128, 128)
                        nc.tensor.matmul(
                            scA[:, j * 512 + dd : (j + 1) * 512],
                            kt_t[0:64, kcol],
                            qTs[hp][c][0:64, ds(dd, 512 - dd)],
                            start=True, stop=True,
                        )
                        nc.tensor.matmul(
                            scB[:, j * 512 + dd : (j + 1) * 512],
                            kt_t[64:128, kcol],
                            qTs[hp][c][64:128, ds(dd, 512 - dd)],
                            start=True, stop=True,
                        )
                    gl = g - 4 * c
                    dd0 = gl * 128
                    dd1 = (gl + 1) * 128
                    for sc_t, pr_t in ((scA, prDA), (scB, prDB)):
                        nc.scalar.activation(
                            out=pr_t[:, ds(gl * 512 + dd0, 512 - dd0)],
                            in_=sc_t[:, dd0:512], func=AF.Exp,
                        )
                        nc.scalar.activation(
                            out=pr_t[:, ds((gl + 1) * 512 + dd1, 512 - dd1)],
                            in_=sc_t[:, 512 + dd1 : 1024], func=AF.Exp,
                        )
                    yield

            def emit_diag_scores(hp, c):
                out = []
                for _ in emit_diag_scores_steps(hp, c, out):
                    pass
                return tuple(out)

            def emit_attention(hp, c, diag=None, pre_pv=None,
                               filler=None):
                def fill(k2):
                    if filler is not None:
                        for _ in range(k2):
                            if next(filler, None) is None:
                                break

                q0 = c * 512
                nkt = 4 * c + 4  # allowed key tiles (block-causal)
                if c > 0:
                    prA = probp.tile([128, 12 * 512], BF16, tag="probs")
                    prB = probp.tile([128, 12 * 512], BF16, tag="probs")
                else:
                    prA = prB = None
                if diag is None:
                    diag = emit_diag_scores(hp, c)
                prDA, prDB = diag
                # causal mask on the 4 diagonal key tiles (DVE bf16 2x),
                # emitted at chunk entry: the diag probs were exp'd during
                # the previous chunk, so DVE masks them while PE runs the
                # clean scores and the pv start tile is ready immediately
                for j in range(4):
                    dd = j * 128
                    for pr in (prDA, prDB):
                        nc.vector.tensor_mul(
                            out=pr[:, ds(j * 512 + dd, 512 - dd)],
                            in0=pr[:, ds(j * 512 + dd, 512 - dd)],
                            in1=masks[:, ds(MOFF[j], 512 - dd)],
                        )
                # clean score groups (transposed: [keys, q]), 2 key tiles
                # per head, one exp per (head, group)
                for g in range(0, 4 * c, 2):
                    scA = ps_sc.tile([128, 1024], F32, tag="sc")
                    scB = ps_sc.tile([128, 1024], F32, tag="sc")
                    for j in (0, 1):
                        kt = g + j
                        kt_t = kTs[hp][kt // 4]
                        kcol = ds((kt % 4) * 128, 128)
                        nc.tensor.matmul(
                            scA[:, j * 512 : (j + 1) * 512],
                            kt_t[0:64, kcol],
                            qTs[hp][c][0:64, :],
                            start=True, stop=True,
                        )
                        nc.tensor.matmul(
                            scB[:, j * 512 : (j + 1) * 512],
                            kt_t[64:128, kcol],
                            qTs[hp][c][64:128, :],
                            start=True, stop=True,
                        )
                    nc.scalar.activation(
                        out=prA[:, ds(g * 512, 1024)],
                        in_=scA[:, :], func=AF.Exp,
                    )
                    nc.scalar.activation(
                        out=prB[:, ds(g * 512, 1024)],
                        in_=scB[:, :], func=AF.Exp,
                    )
                    if g >= 2:
                        fill(1)
                if c != 1:
                    fill(2)
                # pipelined next-chunk diagonal: first group before this
                # chunk's pv, second between the pv half-blocks
                if pre_pv is not None:
                    diag_out, diag_gen = pre_pv()
                    next(diag_gen, None)
                else:
                    diag_out = diag_gen = None
                # pv: rows 0:64 = v.T @ probs, row 64 = denom/SA
                held = []
                for hl, pr, prD in ((0, prA, prDA), (1, prB, prDB)):
                    pv = ps_pv.tile([128, 512], F32, tag="pvpo")
                    kt_order = [4 * c] + list(range(0, 4 * c)) + list(
                        range(4 * c + 1, nkt)
                    )
                    for ki, kt in enumerate(kt_order):
                        if kt < 4 * c:
                            rhs = pr[:, ds(kt * 512, 512)]
                            osl = pv[0:65, 0:512]
                        else:
                            j = kt - 4 * c
                            dd = j * 128
                            rhs = prD[:, ds(j * 512 + dd, 512 - dd)]
                            osl = pv[0:65, dd:512]
                        nc.tensor.matmul(
                            osl,
                            vs[hp][kt // 4][:, ds((kt % 4) * 130 + hl * 65, 65)],
                            rhs,
                            start=(ki == 0),
                            stop=(ki == nkt - 1),
                        )
                    # recip the denom row, broadcast across 64 partitions
                    # via a step-0 partition-source DMA; the whole
                    # normalize chain stays off PE
                    rec = smallp.tile([128, 512], F32, tag="rec")
                    nc.vector.reciprocal(out=rec[64:65, :], in_=pv[64:65, :])
                    bca = smallp.tile([64, 512], F32, tag="bca")
                    nc.sync.dma_start(
                        out=bca[:, :],
                        in_=rec[64:65, :]
                        .rearrange("p (a c) -> p a c", a=1)
                        .broadcast_to((1, 64, 512)),
                    )
                    held.append((hl, pv, bca))
                    if hl == 0 and diag_gen is not None:
                        next(diag_gen, None)
                    if hl == 0:
                        fill(3)
                for hl, pv, bca in held:
                    # multiply straight from psum (the pv tile would idle
                    # through the next chunk's scores phase anyway)
                    if hl == 0:
                        nc.vector.tensor_mul(
                            out=aTc[c][0:64, ds(hp * 512, 512)],
                            in0=pv[0:64, :],
                            in1=bca[:, :],
                        )
                    else:
                        ntmp = smallp.tile([64, 512], F16, tag="ntmp")
                        nc.vector.tensor_mul(
                            out=ntmp[:, :], in0=pv[0:64, :], in1=bca[:, :]
                        )
                        nc.sync.dma_start(
                            out=aTc[c][64:128, ds(hp * 512, 512)],
                            in_=ntmp[:, :],
                        )
                if diag_gen is not None:
                    for _ in diag_gen:
                        pass
                    return tuple(diag_out)
                return None

            # Emission order = PE execution order (in-order queue).  The
            # next head pair's qkv groups are spread into the current
            # attention's chunk windows (coarse, after each chunk: finer
            # interleaving inserts latency into the scores->exp->pv chain
            # and measures worse) so their DVE evictions never burst at a
            # phase boundary and PE always has independent matmul work
            # while ACT churns exp.
            emit_qk0()
            for g in range(4):
                emit_vgrp(0, g)
            chunks = [(hp, c) for hp in range(4) for c in range(4)]
            diag = None
            for ci, (hp, c) in enumerate(chunks):
                    nxt = chunks[ci + 1] if ci + 1 < len(chunks) else None
                    def make_pre(hp2, c2):
                        def pre():
                            out = []
                            return out, emit_diag_scores_steps(hp2, c2, out)
                        return pre
                    pre = (
                        make_pre(nxt[0], nxt[1]) if nxt is not None else None
                    )
                    if hp == 3 and c >= 1:
                        s3f = emit_stage3_steps(c - 1)
                    else:
                        s3f = None
                    diag = emit_attention(hp, c, diag=diag, pre_pv=pre,
                                          filler=s3f)
                    if s3f is not None:
                        for _ in s3f:
                            pass
                    if hp < 3:
                        nxt = hp + 1
                        if c == 0:
                            for n in range(4):
                                ps = ps1.tile([128, 512], F32, tag="ps")
                                emit_qk_group(
                                    nxt, wq8t, rwq8t, qTs[nxt], n, ps
                                )
                        elif c == 1:
                            for n in range(4):
                                ps = ps1.tile([128, 512], F32, tag="ps")
                                emit_qk_group(
                                    nxt, wk8t, rwk8t, kTs[nxt], n, ps
                                )
                        elif c == 2:
                            emit_vgrp(nxt, 0)
                            emit_vgrp(nxt, 1)
                        else:
                            emit_vgrp(nxt, 2)
                            emit_vgrp(nxt, 3)

            emit_stage3(3)

    nc.compile()
    return nc


def _f8(a):
    import ml_dtypes

    return np.asarray(a, np.float32).astype(ml_dtypes.float8_e4m3)


def make_in_maps(x, w_attn, b_attn, w_proj, b_proj):
    """Build the 8 per-core input maps (core 2b+g: batch b, heads 8g..8g+8)."""
    x = np.asarray(x, np.float32)
    w_attn = np.asarray(w_attn, np.float32) * WS
    w_proj = np.asarray(w_proj, np.float32)

    in_maps = []
    for core in range(N_CORES):
        b, g = core // 2, core % 2
        c0 = g * GD
        wq = w_attn[:, c0 : c0 + GD]
        wk = w_attn[:, D + c0 : D + c0 + GD]
        wv = w_attn[:, 2 * D + c0 : 2 * D + c0 + GD]
        wp = w_proj[c0 : c0 + GD, :]
        xT = np.ascontiguousarray(x[b].T)
        import ml_dtypes
        moff = [0, 512, 896, 1152]
        masks = np.zeros((128, 1280), np.float32)
        for j in range(4):
            w = 512 - j * 128
            i_idx = np.arange(128)[:, None]
            q_idx = np.arange(w)[None, :]
            masks[:, moff[j] : moff[j] + w] = (q_idx >= i_idx)
        masks = masks.astype(ml_dtypes.bfloat16)
        x8 = _f8(xT)
        wq8 = _f8(wq)
        wk8 = _f8(wk)
        wv8 = _f8(wv)
        in_maps.append(
            {
                "x8": x8,
                "rx8": _f8(xT - x8.astype(np.float32)),
                "wq8": wq8,
                "rwq8": _f8(wq - wq8.astype(np.float32)),
                "wk8": wk8,
                "rwk8": _f8(wk - wk8.astype(np.float32)),
                "wv8": wv8,
                "rwv8": _f8(wv - wv8.astype(np.float32)),
                "wp": wp.astype(np.float16),
                "masks": masks,
            }
        )
    return in_maps


_CACHED_NC = None


def _reference_fallback(x, w_attn, b_attn, w_proj, b_proj):
    """Plain numpy path for inputs the fast device kernel doesn't cover
    (nonzero biases).  Never hit by the harness (biases are zeros)."""
    x = np.asarray(x, np.float64)
    B, S_, D_ = x.shape
    qkv = x @ np.asarray(w_attn, np.float64) + np.asarray(b_attn, np.float64)
    q, k, v = np.split(qkv, 3, axis=-1)
    H_, HD_ = 16, D_ // 16
    q = q.reshape(B, S_, H_, HD_).transpose(0, 2, 1, 3)
    k = k.reshape(B, S_, H_, HD_).transpose(0, 2, 1, 3)
    v = v.reshape(B, S_, H_, HD_).transpose(0, 2, 1, 3)
    w = np.einsum("bhqd,bhkd->bhqk", q, k)
    mask = np.tril(np.ones((S_, S_)))
    w = w * mask + (-10000.0) * (1.0 - mask)
    w = w - w.max(-1, keepdims=True)
    w = np.exp(w)
    w = w / w.sum(-1, keepdims=True)
    a = np.einsum("bhqk,bhkd->bhqd", w, v)
    a = a.transpose(0, 2, 1, 3).reshape(B, S_, D_)
    return (a @ np.asarray(w_proj, np.float64) + np.asarray(b_proj, np.float64)).astype(
        np.float32
    )


def kernel(x, w_attn, b_attn, w_proj, b_proj, _trace=False):
    global _CACHED_NC
    if np.any(np.asarray(b_attn)) or np.any(np.asarray(b_proj)):
        return _reference_fallback(x, w_attn, b_attn, w_proj, b_proj)

    from concourse.bass_utils import run_bass_kernel_spmd

    if _CACHED_NC is None:
        _CACHED_NC = build_bass()
    nc = _CACHED_NC

    in_maps = make_in_maps(x, w_attn, b_attn, w_proj, b_proj)
    res = run_bass_kernel_spmd(
        nc, in_maps, core_ids=list(range(N_CORES)), trace=_trace
    )
    outs = [np.asarray(r["out"], np.float32) for r in res.results]
    B = np.asarray(x).shape[0]
    full = np.empty((B, S, D), np.float32)
    for b in range(B):
        full[b] = outs[2 * b] + outs[2 * b + 1]
    kernel.last_result = res
    return full


# revision 68
# speedup vs baseline: 1.0214x; 1.0214x over previous
"""Causal multi-head attention (B=4, S=2048, D=1024, H=16) on 8 TRN2 cores.

Sharding (DP on batch x TP on heads): core 2b+g handles batch b and heads
8g..8g+8.  Each core computes the qkv projection for its head group, causal
attention, and a partial output projection; the host sums the two partials
per batch and undoes the fixed power-of-2 scaling -- no device collectives.

v2: fp8 DoubleRow matmuls for the qkv projections.  The qkv weights are
pre-scaled by 256 and shipped as fp8e4 (e4m3) hi/lo pairs; x ships as an
fp8e4 hi/lo pair (x8 + rx8 residual).  q/k/v are computed with a 3-pass
compensated fp8 product (x8*w8 + rx8*w8 + x8*rw8, all DoubleRow with two
128-deep contraction planes per instruction) which matches fp16 precision
at 3/4 (q/k) and 3/4 (v) of the fp16 PE cost.  Scores and pv stay
fp16/bf16 (fp8 probs would need flash-style per-query max subtraction:
unnormalized exp(s) reaches e^18, far past e4m3's 240 max), and the
out-projection stays fp16 (the fp8 quantization of on-device a cannot be
compensated without extra DVE passes and measures 4e-2 max-rel-err).

Scale bookkeeping costs nothing: q/k/v evictions fold 1/256 into the DVE
psum->sbuf copy that already existed (biases are zero, so the bias-add
becomes a scalar-mul of the same cost).  The out-projection is evicted
as bf16 (halves the output DMA bytes) and the host sums/upcasts the two
per-batch partials.

v3 scheduling (cost-model driven, PE busy ~210us of 239us total): the PE
executes its in-order stream, so emission order is the schedule.  The hp0
qkv runs as a 6-group split-pass ramp (pass/j-major, matched to the
serial-DMA arrival order of the fp8 inputs); each later head pair's qkv
groups are emitted into the previous head pair's attention chunk windows;
and each chunk's DIAGONAL score group + exp is emitted inside the
previous chunk (just before its pv) so the pv never waits on ACT and the
exp pipeline always runs one chunk ahead.  The causal mask constant is
DMA'd from the host (gpsimd affine_select would cost ~23us of Pool
time), stage3 uses ps1 during attention and fans out over all psum pools
for the final chunk, whose evictions go to the by-then-idle ACT engine.

Everything else follows the v1 design: layouts avoid all on-device
transposes, block-causal skipping at 128 granularity with column-sliced
diagonal tiles, exp without max-subtraction, denominator via the pv ones
column, reciprocal+broadcast-DMA normalize chain off the PE critical path,
qkv/attention interleaving per head pair, and PSUM-bank phase borrowing
during the startup DMA ramp.
"""

import numpy as np

import concourse.bass as bass
import concourse.mybir as mybir
from concourse import bacc
from concourse.bass import ds
from concourse.tile import TileContext

F16 = mybir.dt.float16
F32 = mybir.dt.float32
BF16 = mybir.dt.bfloat16
F8 = mybir.dt.float8e4

S = 2048  # sequence length
D = 1024  # model dim
HD = 64  # head dim
HPC = 8  # heads per core
GD = HPC * HD  # 512, per-core qkv width
N_CORES = 8

WS = 256.0  # weight prescale (fp8 dynamic range)

AF = mybir.ActivationFunctionType
ALU = mybir.AluOpType
DR = mybir.MatmulPerfMode.DoubleRow


def build_bass(nloop=1):
    nc = bacc.Bacc(None, target_bir_lowering=False)

    x8_d = nc.dram_tensor("x8", [D, S], F8, kind="ExternalInput")
    rx8_d = nc.dram_tensor("rx8", [D, S], F8, kind="ExternalInput")
    wq8_d = nc.dram_tensor("wq8", [D, GD], F8, kind="ExternalInput")
    rwq8_d = nc.dram_tensor("rwq8", [D, GD], F8, kind="ExternalInput")
    wk8_d = nc.dram_tensor("wk8", [D, GD], F8, kind="ExternalInput")
    rwk8_d = nc.dram_tensor("rwk8", [D, GD], F8, kind="ExternalInput")
    wv8_d = nc.dram_tensor("wv8", [D, GD], F8, kind="ExternalInput")
    rwv8_d = nc.dram_tensor("rwv8", [D, GD], F8, kind="ExternalInput")
    wp_d = nc.dram_tensor("wp", [GD, D], F16, kind="ExternalInput")
    masks_d = nc.dram_tensor("masks", [128, 1280], BF16, kind="ExternalInput")
    out_d = nc.dram_tensor("out", [S, D], BF16, kind="ExternalOutput")

    with TileContext(nc) as tc:
     for _loop in range(nloop):
      with tc.tile_pool(name="persist", bufs=1) as persist:
        # Per-head-pair q/k (transposed [douts, rows]; partitions 0:64 =
        # even head dims, 64:128 = odd head dims) and v (natural [keys,
        # per-pair 2*65] with a ones column per head at local col 64 so the
        # pv matmul emits the softmax denominator as row 64).
        qTs, kTs, vs = [], [], []
        for hp in range(4):
            qrow, krow = [], []
            for n in range(4):
                t_q = persist.tile([128, 512], F16, tag=f"qT{hp}_{n}")
                t_k = persist.tile([128, 512], F16, tag=f"kT{hp}_{n}")
                qrow.append(t_q)
                krow.append(t_k)
            vrow = []
            for g in range(4):
                t_v = persist.tile([128, 4 * 130], BF16, tag=f"v{hp}_{g}")
                vrow.append(t_v)
            qTs.append(qrow)
            kTs.append(krow)
            vs.append(vrow)
        wp_sb = persist.tile([128, 4 * D], F16)

        # aT per-chunk tiles: aTc[c] = [128, 4*512], columns hp-major
        # (hp*512 + q-within-chunk); partitions = head-pair dm packing
        aTc = []
        for c in range(4):
            aTc_t = persist.tile([128, 4 * 512], F16, tag=f"aTc{c}")
            aTc.append(aTc_t)

        # Precomputed causal mask tiles (host-built constant: gpsimd
        # affine_select costs ~23us of in-order Pool time and delays every
        # gpsimd-issued DMA behind it): for diagonal offset d = j*128 only
        # columns [d:512) are ever used, and in that sliced frame the
        # triangle is mask[i, qq] = 1 if qq >= i else 0.
        MOFF = [0, 512, 896, 1152]  # packed offsets, widths 512-128j
        masks = persist.tile([128, 1280], BF16)

        with (
            tc.tile_pool(name="stage1", bufs=1) as s1,
            tc.tile_pool(name="probs", bufs=3) as probp,
            tc.tile_pool(name="small", bufs=2) as smallp,
            tc.tile_pool(name="outp", bufs=8) as outp,
            tc.tile_pool(name="ps1", bufs=2, space="PSUM") as ps1,
            tc.tile_pool(name="ps_sc", bufs=2, space="PSUM") as ps_sc,
            tc.tile_pool(name="ps_pv", bufs=2, space="PSUM") as ps_pv,
        ):
            # Input tiles hold DoubleRow kt-plane pairs: tile j's columns
            # [0:W] are contraction rows 2j*128..2j*128+127, [W:2W] are rows
            # (2j+1)*128...  DMAs split across the sync/scalar/gpsimd issue
            # queues, ordered by first use: wq8 + x8 first (pass A of the
            # first q groups), then the residuals, then wk8/wv8/wp8.
            def pair_load(dram, j, width, tile, eng):
                eng.dma_start(
                    out=tile[:, :].rearrange("p (two s) -> p two s", two=2),
                    in_=dram[2 * j * 128 : (2 * j + 2) * 128, :].rearrange(
                        "(two p) s -> p two s", two=2
                    ),
                )

            # arrival order tuned to the hp0 split-pass ramp: w tensors +
            # x8 first (A/C passes), rx8 (B) after, then the v/proj
            # weights.  Each weight tensor is a single DMA (one
            # descriptor-generation stage instead of four -- the dge
            # stages serialize and dominate the small w transfers); x8
            # rides the SWDGE queue so its dge doesn't queue behind the
            # weights' HWDGE stages.
            def w_load(dram, name, eng):
                W = dram.shape[1]
                t = s1.tile([128, 8 * W], F8, tag=name, name=name + "t")
                eng.dma_start(
                    out=t[:, :].rearrange(
                        "p (four two c) -> p four two c", four=4, two=2
                    ),
                    in_=dram[:, :].rearrange(
                        "(four two p) c -> p four two c", four=4, two=2
                    ),
                )
                return t

            sy, sc_, gp = nc.sync, nc.scalar, nc.gpsimd
            # issue interleaved x8[j]/wq8[j] pairs so the serial DMA device
            # delivers them in the exact order the hp0 A-pass consumes them
            wq8t = w_load(wq8_d, "wq8", sy)
            rwq8t = w_load(rwq8_d, "rwq8", sy)
            wk8t = w_load(wk8_d, "wk8", sy)
            rwk8t = w_load(rwk8_d, "rwk8", sy)
            x8t, rx8t = [None] * 4, [None] * 4
            for j in range(4):
                t = s1.tile([128, 2 * S], F8, tag=f"x8{j}", name=f"x8s{j}")
                pair_load(x8_d, j, S, t, gp)
                x8t[j] = t
            for j in range(4):
                t = s1.tile([128, 2 * S], F8, tag=f"rx8{j}", name=f"rx8s{j}")
                pair_load(rx8_d, j, S, t, sc_)
                rx8t[j] = t
            nc.gpsimd.dma_start(out=masks[:, :], in_=masks_d[:, :])
            wv8t = w_load(wv8_d, "wv8", gp)
            rwv8t = w_load(rwv8_d, "rwv8", gp)
            for k in range(4):
                nc.sync.dma_start(
                    out=wp_sb[:, ds(k * D, D)],
                    in_=wp_d[k * 128 : (k + 1) * 128, :],
                )

            def w_planes(t, j, hp):
                return t[:, :].rearrange(
                    "p (four two c) -> p four two c", four=4, two=2
                )[:, j, :, ds(hp * 128, 128)]

            def x_planes(tiles, j, c0, w):
                return tiles[j][:, :].rearrange(
                    "p (two s) -> p two s", two=2
                )[:, :, ds(c0, w)]

            # one qkv projection group: 3-pass compensated fp8 DoubleRow
            # (A: x8*w8, C: x8*rw8, B: rx8*w8 -- B last so the rx8 DMAs
            # are off the startup critical path).
            def emit_qk_group(hp, wt, rwt, dst, n, ps):
                passes = ((wt, x8t), (rwt, x8t), (wt, rx8t))
                for pi, (lw, lx) in enumerate(passes):
                    for j in range(4):
                        nc.tensor.matmul(
                            ps[:, :],
                            w_planes(lw, j, hp),
                            x_planes(lx, j, n * 512, 512),
                            start=(pi == 0 and j == 0),
                            stop=(pi == 2 and j == 3),
                            perf_mode=DR,
                        )
                nc.vector.tensor_scalar_mul(
                    out=dst[n][:, :], in0=ps[:, :], scalar1=1.0 / WS
                )

            # hp0 startup: 6 psum groups held open (q n0..3 + k n0,n1,
            # borrowing the idle scores/pv banks), emitted pass-major and
            # j-major so each arriving DMA tile unlocks a batch of matmuls
            # and PE ramps as the inputs trickle in.
            def emit_qk0():
                qps = [
                    ps1.tile([128, 512], F32, tag="ps", name="qps0"),
                    ps1.tile([128, 512], F32, tag="ps", name="qps1"),
                    ps_sc.tile([128, 512], F32, tag="sc", name="qps2"),
                    ps_sc.tile([128, 512], F32, tag="sc", name="qps3"),
                ]
                # p-state warmers: zero-data K=1 matmuls through the pv
                # psum slots keep the PE ramp alive through the startup
                # DMA wait, so the real split-pass matmuls run at the full
                # 2.4GHz cycle instead of the mid p-state.
                zf = smallp.tile([1, 512], F16, name="zf")
                nc.vector.memset(zf[:, :], 0.0)
                fill_tiles = [
                    ps_pv.tile([128, 512], F32, tag="pvpo", name=f"fill{i}")
                    for i in range(26)
                ]
                fill_n = [0]

                def emit_fill(count):
                    for _ in range(count):
                        if fill_n[0] >= len(fill_tiles):
                            return
                        nc.tensor.matmul(
                            fill_tiles[fill_n[0]][:, :],
                            zf[0:1, 0:128],
                            zf[0:1, 0:512],
                            start=True, stop=True,
                        )
                        fill_n[0] += 1

                emit_fill(14)
                kps = [
                    ps_pv.tile([128, 512], F32, tag="pvpo", name="kps0"),
                    ps_pv.tile([128, 512], F32, tag="pvpo", name="kps1"),
                ]
                groups = [("q", n, qps[n]) for n in range(4)] + [
                    ("k", n, kps[n]) for n in range(2)
                ]

                def w_of(kind, res):
                    if kind == "q":
                        return rwq8t if res else wq8t
                    return rwk8t if res else wk8t

                qg = [g for g in groups if g[0] == "q"]
                kg = [g for g in groups if g[0] == "k"]
                sched = [s for j in range(4)
                         for s in ((0, False, x8t, j, qg),
                                   (1, True, x8t, j, qg))]
                sched += [s for j in range(4)
                          for s in ((0, False, x8t, j, kg),
                                    (1, True, x8t, j, kg))]
                sched += [(2, False, rx8t, j, qg) for j in range(4)]
                sched += [(2, False, rx8t, j, kg) for j in range(4)]
                for pi, res, lx, j, gset in sched:
                    for kind, n, ps in gset:
                        nc.tensor.matmul(
                            ps[:, :],
                            w_planes(w_of(kind, res), j, 0),
                            x_planes(lx, j, n * 512, 512),
                            start=(pi == 0 and j == 0),
                            stop=(pi == 2 and j == 3),
                            perf_mode=DR,
                        )
                    if pi != 2:
                        emit_fill(2)
                emit_fill(len(fill_tiles))
                for kind, n, ps in groups:
                    dst = qTs[0] if kind == "q" else kTs[0]
                    nc.vector.tensor_scalar_mul(
                        out=dst[n][:, :], in0=ps[:, :], scalar1=1.0 / WS
                    )
                for n in (2, 3):
                    ps = ps1.tile([128, 512], F32, tag="ps")
                    emit_qk_group(0, wk8t, rwk8t, kTs[0], n, ps)

            def emit_vgrp(hp, g):
                # v rows for key tiles 4g..4g+3 of head pair hp (3-pass
                # compensated fp8 DoubleRow); eviction folds the 1/WS
                # descale.
                for rl in range(4):
                    rt = 4 * g + rl
                    ps = ps1.tile([128, 512], F32, tag="ps")
                    passes = ((x8t, wv8t), (rx8t, wv8t), (x8t, rwv8t))
                    for pi, (lx, lw) in enumerate(passes):
                        for j in range(4):
                            nc.tensor.matmul(
                                ps[0:128, 0:128],
                                x_planes(lx, j, rt * 128, 128),
                                w_planes(lw, j, hp),
                                start=(pi == 0 and j == 0),
                                stop=(pi == 2 and j == 3),
                                perf_mode=DR,
                            )
                    # interleaved store: local head hl -> cols
                    # [hl*65, hl*65+64)
                    out_ap = vs[hp][g][:, ds(rl * 130, 130)].rearrange(
                        "p (h c) -> p h c", h=2
                    )[:, :, 0:64]
                    in_ap = ps[:, 0:128].rearrange("p (h c) -> p h c", h=2)
                    nc.vector.tensor_scalar_mul(
                        out=out_ap, in0=in_ap, scalar1=1.0 / WS
                    )
                # ones columns (softmax denominator source)
                ones_ap = vs[hp][g][:, :].rearrange("p (r c) -> p r c", c=65)[
                    :, :, 64:65
                ]
                nc.gpsimd.memset(ones_ap, 1.0)

            # out-projection for one chunk-column (all 4 aTc[c] writers
            # done); bias is zero so the eviction is a plain copy.  Both
            # psum groups come from ps1 (free during hp3's attention) so
            # stage3 never contends with the attention pv pool; the final
            # chunk's evictions go to ACT (idle once the last exps retire)
            # so the tail doesn't serialize behind DVE.
            def emit_stage3_steps(c3):
                for rt in range(4 * c3, 4 * c3 + 4):
                    for nch in range(2):
                        if c3 == 3:
                            # attention is done: every psum pool is idle
                            k3 = (2 * (rt % 4) + nch) % 3
                            if k3 == 0:
                                ps = ps1.tile([128, 512], F32, tag="ps")
                            elif k3 == 1:
                                ps = ps_pv.tile([128, 512], F32, tag="pvpo")
                            else:
                                ps = ps_sc.tile([128, 512], F32, tag="sc")
                        else:
                            ps = ps1.tile([128, 512], F32, tag="ps")
                        for kt4 in range(4):
                            nc.tensor.matmul(
                                ps[:, :],
                                aTc[c3][:, ds(kt4 * 512 + (rt % 4) * 128, 128)],
                                wp_sb[:, ds(kt4 * D + nch * 512, 512)],
                                start=(kt4 == 0),
                                stop=(kt4 == 3),
                            )
                        osb = outp.tile([128, 512], BF16, tag="osb")
                        if c3 == 3:
                            nc.scalar.copy(out=osb[:, :], in_=ps[:, :])
                        else:
                            nc.vector.tensor_copy(out=osb[:, :], in_=ps[:, :])
                        nc.sync.dma_start(
                            out=out_d[
                                rt * 128 : (rt + 1) * 128,
                                nch * 512 : (nch + 1) * 512,
                            ],
                            in_=osb[:, :],
                        )
                        yield

            def emit_stage3(c3):
                for _ in emit_stage3_steps(c3):
                    pass

            # attention for (hp, c). Even head on PE row-tile (0,0), odd head
            # on (64,0); adjacent even/odd matmuls run concurrently on the
            # two array halves.
            def emit_diag_scores_steps(hp, c, out):
                # the 2 diagonal score groups + exps of chunk (hp, c);
                # emitted from inside the PREVIOUS chunk (first group just
                # before its pv, second between the pv half-blocks so the
                # sc-pool recycle overlaps pv matmuls) -- ACT gets a head
                # start and this chunk's pv never waits on its first probs
                q0 = c * 512
                prDA = probp.tile([128, 4 * 512], BF16, tag="probsD",
                                  bufs=4)
                prDB = probp.tile([128, 4 * 512], BF16, tag="probsD",
                                  bufs=4)
                out.extend([prDA, prDB])
                for g in (4 * c, 4 * c + 2):
                    scA = ps_sc.tile([128, 1024], F32, tag="sc")
                    scB = ps_sc.tile([128, 1024], F32, tag="sc")
                    for j in (0, 1):
                        kt = g + j
                        dd = max(0, kt * 128 - q0)
                        kt_t = kTs[hp][kt // 4]
                        kcol = ds((kt % 4) * 128, 128)
                        nc.tensor.matmul(
                            scA[:, j * 512 + dd : (j + 1) * 512],
                            kt_t[0:64, kcol],
                            qTs[hp][c][0:64, ds(dd, 512 - dd)],
                            start=True, stop=True,
                        )
                        nc.tensor.matmul(
                            scB[:, j * 512 + dd : (j + 1) * 512],
                            kt_t[64:128, kcol],
                            qTs[hp][c][64:128, ds(dd, 512 - dd)],
                            start=True, stop=True,
                        )
                    gl = g - 4 * c
                    dd0 = gl * 128
                    dd1 = (gl + 1) * 128
                    for sc_t, pr_t in ((scA, prDA), (scB, prDB)):
                        nc.scalar.activation(
                            out=pr_t[:, ds(gl * 512 + dd0, 512 - dd0)],
                            in_=sc_t[:, dd0:512], func=AF.Exp,
                        )
                        nc.scalar.activation(
                            out=pr_t[:, ds((gl + 1) * 512 + dd1, 512 - dd1)],
                            in_=sc_t[:, 512 + dd1 : 1024], func=AF.Exp,
                        )
                    yield

            def emit_diag_scores(hp, c):
                out = []
                for _ in emit_diag_scores_steps(hp, c, out):
                    pass
                return tuple(out)

            def emit_attention(hp, c, diag=None, pre_pv=None,
                               filler=None):
                def fill(k2):
                    if filler is not None:
                        for _ in range(k2):
                            if next(filler, None) is None:
                                break

                q0 = c * 512
                nkt = 4 * c + 4  # allowed key tiles (block-causal)
                if c > 0:
                    prA = probp.tile([128, 12 * 512], BF16, tag="probs")
                    prB = probp.tile([128, 12 * 512], BF16, tag="probs")
                else:
                    prA = prB = None
                if diag is None:
                    diag = emit_diag_scores(hp, c)
                prDA, prDB = diag
                # causal mask on the 4 diagonal key tiles (DVE bf16 2x),
                # emitted at chunk entry: the diag probs were exp'd during
                # the previous chunk, so DVE masks them while PE runs the
                # clean scores and the pv start tile is ready immediately
                for j in range(4):
                    dd = j * 128
                    for pr in (prDA, prDB):
                        nc.vector.tensor_mul(
                            out=pr[:, ds(j * 512 + dd, 512 - dd)],
                            in0=pr[:, ds(j * 512 + dd, 512 - dd)],
                            in1=masks[:, ds(MOFF[j], 512 - dd)],
                        )
                # clean score groups (transposed: [keys, q]), 2 key tiles
                # per head, one exp per (head, group)
                for g in range(0, 4 * c, 2):
                    scA = ps_sc.tile([128, 1024], F32, tag="sc")
                    scB = ps_sc.tile([128, 1024], F32, tag="sc")
                    for j in (0, 1):
                        kt = g + j
                        kt_t = kTs[hp][kt // 4]
                        kcol = ds((kt % 4) * 128, 128)
                        nc.tensor.matmul(
                            scA[:, j * 512 : (j + 1) * 512],
                            kt_t[0:64, kcol],
                            qTs[hp][c][0:64, :],
                            start=True, stop=True,
                        )
                        nc.tensor.matmul(
                            scB[:, j * 512 : (j + 1) * 512],
                            kt_t[64:128, kcol],
                            qTs[hp][c][64:128, :],
                            start=True, stop=True,
                        )
                    nc.scalar.activation(
                        out=prA[:, ds(g * 512, 1024)],
                        in_=scA[:, :], func=AF.Exp,
                    )
                    nc.scalar.activation(
                        out=prB[:, ds(g * 512, 1024)],
                        in_=scB[:, :], func=AF.Exp,
                    )
                fill(3)
                # pipelined next-chunk diagonal: first group before this
                # chunk's pv, second between the pv half-blocks
                if pre_pv is not None:
                    diag_out, diag_gen = pre_pv()
                    next(diag_gen, None)
                else:
                    diag_out = diag_gen = None
                # pv: rows 0:64 = v.T @ probs, row 64 = denom/SA
                held = []
                for hl, pr, prD in ((0, prA, prDA), (1, prB, prDB)):
                    pv = ps_pv.tile([128, 512], F32, tag="pvpo")
                    kt_order = [4 * c] + list(range(0, 4 * c)) + list(
                        range(4 * c + 1, nkt)
                    )
                    for ki, kt in enumerate(kt_order):
                        if kt < 4 * c:
                            rhs = pr[:, ds(kt * 512, 512)]
                            osl = pv[0:65, 0:512]
                        else:
                            j = kt - 4 * c
                            dd = j * 128
                            rhs = prD[:, ds(j * 512 + dd, 512 - dd)]
                            osl = pv[0:65, dd:512]
                        nc.tensor.matmul(
                            osl,
                            vs[hp][kt // 4][:, ds((kt % 4) * 130 + hl * 65, 65)],
                            rhs,
                            start=(ki == 0),
                            stop=(ki == nkt - 1),
                        )
                    # recip the denom row, broadcast across 64 partitions
                    # via a step-0 partition-source DMA; the whole
                    # normalize chain stays off PE
                    rec = smallp.tile([128, 512], F32, tag="rec")
                    nc.vector.reciprocal(out=rec[64:65, :], in_=pv[64:65, :])
                    bca = smallp.tile([64, 512], F32, tag="bca")
                    nc.sync.dma_start(
                        out=bca[:, :],
                        in_=rec[64:65, :]
                        .rearrange("p (a c) -> p a c", a=1)
                        .broadcast_to((1, 64, 512)),
                    )
                    held.append((hl, pv, bca))
                    if hl == 0 and diag_gen is not None:
                        next(diag_gen, None)
                    if hl == 0:
                        fill(3)
                for hl, pv, bca in held:
                    # multiply straight from psum (the pv tile would idle
                    # through the next chunk's scores phase anyway)
                    if hl == 0:
                        nc.vector.tensor_mul(
                            out=aTc[c][0:64, ds(hp * 512, 512)],
                            in0=pv[0:64, :],
                            in1=bca[:, :],
                        )
                    else:
                        ntmp = smallp.tile([64, 512], F16, tag="ntmp")
                        nc.vector.tensor_mul(
                            out=ntmp[:, :], in0=pv[0:64, :], in1=bca[:, :]
                        )
                        nc.sync.dma_start(
                            out=aTc[c][64:128, ds(hp * 512, 512)],
                            in_=ntmp[:, :],
                        )
                if diag_gen is not None:
                    for _ in diag_gen:
                        pass
                    return tuple(diag_out)
                return None

            # Emission order = PE execution order (in-order queue).  The
            # next head pair's qkv groups are spread into the current
            # attention's chunk windows (coarse, after each chunk: finer
            # interleaving inserts latency into the scores->exp->pv chain
            # and measures worse) so their DVE evictions never burst at a
            # phase boundary and PE always has independent matmul work
            # while ACT churns exp.
            emit_qk0()
            for g in range(4):
                emit_vgrp(0, g)
            chunks = [(hp, c) for hp in range(4) for c in range(4)]
            diag = None
            for ci, (hp, c) in enumerate(chunks):
                    nxt = chunks[ci + 1] if ci + 1 < len(chunks) else None
                    def make_pre(hp2, c2):
                        def pre():
                            out = []
                            return out, emit_diag_scores_steps(hp2, c2, out)
                        return pre
                    pre = (
                        make_pre(nxt[0], nxt[1]) if nxt is not None else None
                    )
                    if hp == 3 and c >= 1:
                        s3f = emit_stage3_steps(c - 1)
                    else:
                        s3f = None
                    diag = emit_attention(hp, c, diag=diag, pre_pv=pre,
                                          filler=s3f)
                    if s3f is not None:
                        for _ in s3f:
                            pass
                    if hp < 3:
                        nxt = hp + 1
                        if c == 0:
                            for n in range(4):
                                ps = ps1.tile([128, 512], F32, tag="ps")
                                emit_qk_group(
                                    nxt, wq8t, rwq8t, qTs[nxt], n, ps
                                )
                        elif c == 1:
                            for n in range(4):
                                ps = ps1.tile([128, 512], F32, tag="ps")
                                emit_qk_group(
                                    nxt, wk8t, rwk8t, kTs[nxt], n, ps
                                )
                        elif c == 2:
                            emit_vgrp(nxt, 0)
                            emit_vgrp(nxt, 1)
                        else:
                            emit_vgrp(nxt, 2)
                            emit_vgrp(nxt, 3)

            emit_stage3(3)

    nc.compile()
    return nc


def _f8(a):
    import ml_dtypes

    return np.asarray(a, np.float32).astype(ml_dtypes.float8_e4m3)


def make_in_maps(x, w_attn, b_attn, w_proj, b_proj):
    """Build the 8 per-core input maps (core 2b+g: batch b, heads 8g..8g+8)."""
    x = np.asarray(x, np.float32)
    w_attn = np.asarray(w_attn, np.float32) * WS
    w_proj = np.asarray(w_proj, np.float32)

    in_maps = []
    for core in range(N_CORES):
        b, g = core // 2, core % 2
        c0 = g * GD
        wq = w_attn[:, c0 : c0 + GD]
        wk = w_attn[:, D + c0 : D + c0 + GD]
        wv = w_attn[:, 2 * D + c0 : 2 * D + c0 + GD]
        wp = w_proj[c0 : c0 + GD, :]
        xT = np.ascontiguousarray(x[b].T)
        import ml_dtypes
        moff = [0, 512, 896, 1152]
        masks = np.zeros((128, 1280), np.float32)
        for j in range(4):
            w = 512 - j * 128
            i_idx = np.arange(128)[:, None]
            q_idx = np.arange(w)[None, :]
            masks[:, moff[j] : moff[j] + w] = (q_idx >= i_idx)
        masks = masks.astype(ml_dtypes.bfloat16)
        x8 = _f8(xT)
        wq8 = _f8(wq)
        wk8 = _f8(wk)
        wv8 = _f8(wv)
        in_maps.append(
            {
                "x8": x8,
                "rx8": _f8(xT - x8.astype(np.float32)),
                "wq8": wq8,
                "rwq8": _f8(wq - wq8.astype(np.float32)),
                "wk8": wk8,
                "rwk8": _f8(wk - wk8.astype(np.float32)),
                "wv8": wv8,
                "rwv8": _f8(wv - wv8.astype(np.float32)),
                "wp": wp.astype(np.float16),
                "masks": masks,
            }
        )
    return in_maps


_CACHED_NC = None


def _reference_fallback(x, w_attn, b_attn, w_proj, b_proj):
    """Plain numpy path for inputs the fast device kernel doesn't cover
    (nonzero biases).  Never hit by the harness (biases are zeros)."""
    x = np.asarray(x, np.float64)
    B, S_, D_ = x.shape
    qkv = x @ np.asarray(w_attn, np.float64) + np.asarray(b_attn, np.float64)
    q, k, v = np.split(qkv, 3, axis=-1)
    H_, HD_ = 16, D_ // 16
    q = q.reshape(B, S_, H_, HD_).transpose(0, 2, 1, 3)
    k = k.reshape(B, S_, H_, HD_).transpose(0, 2, 1, 3)
    v = v.reshape(B, S_, H_, HD_).transpose(0, 2, 1, 3)
    w = np.einsum("bhqd,bhkd->bhqk", q, k)
    mask = np.tril(np.ones((S_, S_)))
    w = w * mask + (-10000.0) * (1.0 - mask)
    w = w - w.max(-1, keepdims=True)
    w = np.exp(w)
    w = w / w.sum(-1, keepdims=True)
    a = np.einsum("bhqk,bhkd->bhqd", w, v)
    a = a.transpose(0, 2, 1, 3).reshape(B, S_, D_)
    return (a @ np.asarray(w_proj, np.float64) + np.asarray(b_proj, np.float64)).astype(
        np.float32
    )


def kernel(x, w_attn, b_attn, w_proj, b_proj, _trace=False):
    global _CACHED_NC
    if np.any(np.asarray(b_attn)) or np.any(np.asarray(b_proj)):
        return _reference_fallback(x, w_attn, b_attn, w_proj, b_proj)

    from concourse.bass_utils import run_bass_kernel_spmd

    if _CACHED_NC is None:
        _CACHED_NC = build_bass()
    nc = _CACHED_NC

    in_maps = make_in_maps(x, w_attn, b_attn, w_proj, b_proj)
    res = run_bass_kernel_spmd(
        nc, in_maps, core_ids=list(range(N_CORES)), trace=_trace
    )
    outs = [np.asarray(r["out"], np.float32) for r in res.results]
    B = np.asarray(x).shape[0]
    full = np.empty((B, S, D), np.float32)
    for b in range(B):
        full[b] = outs[2 * b] + outs[2 * b + 1]
    kernel.last_result = res
    return full
